# revision 8
# baseline (speedup 1.0000x reference)
"""DRAW-style read attention on Trainium2 — data-parallel over batch on 8 NeuronCores.

reference math (per batch element):
    params = h @ W.T + b                         [5]
    g_x = 64.5*(p0+1)-0.5 ; g_y likewise
    sigma2 = exp(p2) ; delta = (127/31)*exp(p3) ; gamma = exp(p4)
    mu_x[n] = g_x + (n-15.5)*delta ; mu_y likewise
    FX[n,h] = exp(-(h-mu_x[n])^2/(2 sigma2)) / (Z_n + 1e-8)    (Z_n = row sum)
    FY[m,w] likewise
    patch_i = FX @ img_i @ FY.T   for img in (x_c0..2, xhat_c0..2)
    out = gamma * flatten(patches)               [6144]

device layout per core (local batch B=32):
    images arrive pre-cast to bf16 in [quad, h, b4, c, w] layout so every DMA
    partition line is a 3KB contiguous DRAM run; the 8 quad DMAs are the FIRST
    instructions on the sync queue so the HBM stream starts as early as
    possible (the kernel is HBM-stream-bound at ~358 GB/s/core)
    the tiny params chain (h @ W'' -> exp -> mu/s/gamma) runs on HOST; the
    device receives the per-(b,n) filterbank coefficients [s, -mu_x*s,
    -mu_y*s, gamma] directly (16KB), so the only ACT table ever loaded is
    erf_derivative and the whole on-device params/expansion pipeline is gone
    filterbanks in [bn, hw] layout: ACT Derivative_Erf(s*grid - mu*s) =
    2/sqrt(pi)*exp(-u^2) gives the gaussian + accum_out Z in one pass;
    normalize on DVE (gamma folded into FY's scale), then transpose to
    FXT/FYT [hw, bn] via REGULAR matmul against a bf16 identity (runs at
    2.4GHz vs 1.2GHz transpose-mode)
    main loop over pairs of b: At[w,n] = img[h,w].T @ FXT_b (12 matmuls/pair)
    then batched mm2: 4 matmuls/pair with At quads as 128/64-col stationary
    weights -> patch psum [128(ic,n), 32(m)]; outputs DMA'd in psum-native
    layout (384B lines, sync queue) and re-flattened on host
"""

import math

import numpy as np

import concourse.bass as bass  # noqa: F401  (import keeps bass registered)
import concourse.mybir as mybir
import concourse.tile as tile
from concourse import bacc
from concourse.bass_utils import run_bass_kernel_spmd
from concourse.masks import make_identity

F32 = mybir.dt.float32
BF16 = mybir.dt.bfloat16

NCORES = 8
B = 32          # per-core batch shard
C = 3
IMG = 128
N = 32
DH = 1024
U = 2 * C       # images per batch element: x channels 0..2 then x_hat channels 0..2
NT = (B * N) // 128   # quads: tiles over the flattened (b, n) axis
NP = B // 2           # batch pairs
DELTA_NORM = (IMG - 1.0) / (N - 1.0)
SQRT_PI_2 = math.sqrt(math.pi) / 2.0


def build_nc(finalize=True):
    nc = bacc.Bacc("TRN2", target_bir_lowering=False, debug=False, num_devices=NCORES)
    AFT = mybir.ActivationFunctionType

    # images pre-cast + pre-laid-out on host: [quad, half, h, i, b2, c, w]
    # bf16 (x and x_hat interleaved; each batch-PAIR is one 3KB-per-line DMA
    # so the main loop tracks the HBM stream at pair granularity)
    x4_d = nc.declare_dram_parameter("x4", [NT * 2, 128, 2 * 2 * C * IMG], BF16,
                                     isOutput=False)
    # per-(b,n) filterbank coefficients [s, -mu_x*s, -mu_y*s, gamma] + grid
    cg_d = nc.declare_dram_parameter("cg", [128, NT * 4 + IMG], F32,
                                     isOutput=False)
    # psum-native output layout, bf16 (host re-flattens + upcasts, see
    # unpack_out; the rel-err budget easily covers one more bf16 rounding)
    out_d = nc.declare_dram_parameter("out", [NT, 128, 2, 3, N], BF16,
                                      isOutput=True)

    with tile.TileContext(nc) as tc:
        with (
            tc.tile_pool(name="consts", bufs=1) as consts,
            tc.tile_pool(name="fb", bufs=4) as fb,
            tc.tile_pool(name="imgb_p", bufs=16) as imgb_p,
            tc.tile_pool(name="atb_p", bufs=3) as atb_p,
            tc.tile_pool(name="outs_p", bufs=3) as outs_p,
            tc.tile_pool(name="ps_tr", bufs=2, space="PSUM") as ps_tr,
            tc.tile_pool(name="ps_at", bufs=3, space="PSUM") as ps_at,
            tc.tile_pool(name="ps_pt", bufs=3, space="PSUM") as ps_pt,
        ):
            # ---- filterbank coefficients FIRST on the sync queue (NOT the
            # scalar queue: the erf_derivative ACT table load is itself a DMA
            # on the scalar HWDGE ring and would push this transfer out by
            # ~3us), then the image stream
            cg_sb = consts.tile([128, NT * 4 + IMG], F32)
            nc.sync.dma_start(out=cg_sb, in_=cg_d[:])
            fbv = cg_sb[:, 0:NT * 4].rearrange("p (t j) -> p t j", t=NT)
            grid_sb = cg_sb[:, NT * 4:]

            def emit_half_dma(hq):
                imgb = imgb_p.tile([128, 2, 2, C, IMG], BF16, tag="imgb",
                                   name=f"imgb{hq}")
                nc.sync.dma_start(out=imgb, in_=x4_d[hq])
                return imgb

            halves = {hq: emit_half_dma(hq) for hq in range(NT * 2)}

            # tiny on-chip consts + ACT table prime while DMAs fly (memset
            # before make_identity so the prime's bias operand is ready
            # immediately and the ~2.9us table load starts ASAP)
            zeros = consts.tile([128, 1], F32)
            nc.vector.memset(zeros, 0.0)
            prime_t = consts.tile([1, 1], F32)
            nc.scalar.activation(prime_t, zeros[:1], AFT.Derivative_Erf,
                                 scale=-1.0, bias=zeros[:1])
            ident = consts.tile([128, 128], BF16)
            make_identity(nc, ident)

            # both filterbanks bf16 (matmuls run bf16); gamma folded into FY;
            # one combined tile so each tile's epilogue is a single DVE cast
            FXY = consts.tile([128, 2, B * N], BF16)
            FXT = FXY[:, 0, :]
            FYT = FXY[:, 1, :]

            def fbank2(t):
                # Derivative_Erf(u) = 2/sqrt(pi) * exp(-u^2) with u =
                # s*grid - s*mu: one ACT per filterbank half, accum_out
                # yields Z for free. The 2/sqrt(pi) factor cancels in
                # F/(Z+eps) once eps is scaled by the same factor.
                e_un = fb.tile([128, 2, IMG], BF16, tag="e_un")
                Z2 = fb.tile([128, 2], F32, tag="Z2")
                nc.scalar.activation(e_un[:, 0, :], grid_sb, AFT.Derivative_Erf,
                                     scale=fbv[:, t, 0:1], bias=fbv[:, t, 1:2],
                                     accum_out=Z2[:, 0:1])
                nc.scalar.activation(e_un[:, 1, :], grid_sb, AFT.Derivative_Erf,
                                     scale=fbv[:, t, 0:1], bias=fbv[:, t, 2:3],
                                     accum_out=Z2[:, 1:2])
                # the reference ADDS eps — for borderline off-grid rows Z is
                # itself ~1e-8, so a max-clamp is NOT equivalent. Startup
                # tiles run the chain on DVE (fewer cross-engine hops);
                # steady-state puts the small ops on the idle gpsimd
                small = nc.vector if t < 2 else nc.gpsimd
                small.tensor_scalar_add(Z2, Z2, 1e-8 / SQRT_PI_2)
                invZ2 = fb.tile([128, 2], F32, tag="invZ2")
                nc.vector.reciprocal_approx_fast(invZ2, Z2)
                small.tensor_mul(invZ2[:, 1:2], invZ2[:, 1:2], fbv[:, t, 3:4])
                ps_t = ps_tr.tile([128, 2, 128], F32, tag="ps_t")
                for j in range(2):
                    Fn = fb.tile([128, IMG], BF16, tag="Fn")
                    nc.vector.tensor_scalar_mul(Fn, e_un[:, j, :],
                                                invZ2[:, j:j + 1])
                    # Fn.T @ I — a regular matmul IS the transpose, and runs
                    # at the warm 2.4GHz clock (transpose-mode stays at 1.2)
                    nc.tensor.matmul(ps_t[:, j, :], Fn, ident,
                                     start=True, stop=True)
                nc.vector.tensor_copy(FXY[:, :, t * 128:(t + 1) * 128], ps_t)

            # ---- main loop: pairs of batch elements; mm2 pipelined one pair
            # behind so the atb copy latency hides under the next mm1 ----
            def mm1(P, imgb):
                ps_a = ps_at.tile([128, 2, U, N], F32, tag="ps_a")
                for b2 in range(2):
                    b = 2 * P + b2
                    for i in range(2):
                        for c in range(C):
                            nc.tensor.matmul(ps_a[:, b2, i * C + c, :],
                                             imgb[:, i, b2, c, :],
                                             FXT[:, b * N:(b + 1) * N],
                                             start=True, stop=True)
                atb = atb_p.tile([128, 2, U, N], BF16, tag="atb")
                nc.vector.tensor_copy(atb, ps_a)
                return atb

            def mm2(P, atb):
                # At quads as 128/64-col stationary weights: 4 matmuls/pair.
                # psum partitions: j0/j1 -> (ic0..3, n) of b0/b1, j2 -> (b2,
                # ic4..5, n) via the 64-col tile at column offset 64.
                # per-pair epilogue: ACT copy + SWDGE out DMA (gpsimd queue,
                # off the HWDGE rings that carry the image stream)
                tq, pp = P // 2, P % 2
                ps_p = ps_pt.tile([128, 3, N], F32, tag="ps_o", name=f"pso{P}")
                b0, b1 = 2 * P, 2 * P + 1
                fy0 = FYT[:, b0 * N:(b0 + 1) * N]
                fy1 = FYT[:, b1 * N:(b1 + 1) * N]
                nc.tensor.matmul(ps_p[:, 0, :], atb[:, 0, 0:4, :], fy0,
                                 start=True, stop=True)
                nc.tensor.matmul(ps_p[:, 1, :], atb[:, 1, 0:4, :], fy1,
                                 start=True, stop=True)
                nc.tensor.matmul(ps_p[0:64, 2, :], atb[:, 0, 4:6, :], fy0,
                                 start=True, stop=True)
                nc.tensor.matmul(ps_p[64:128, 2, :], atb[:, 1, 4:6, :], fy1,
                                 start=True, stop=True, tile_position=(0, 64))
                outs = outs_p.tile([128, 3, N], BF16, tag="outs")
                nc.scalar.copy(outs, ps_p)
                nc.gpsimd.dma_start(out=out_d[tq][:, pp], in_=outs)

            fbank2(0)
            fbank2(1)
            prev = None
            fbank2(2)
            for t in range(NT):
                # filterbanks three tiles ahead
                if t + 3 < NT:
                    fbank2(t + 3)
                for pp in range(2):
                    P = 2 * t + pp
                    atb = mm1(P, halves.pop(P))
                    if prev is not None:
                        mm2(*prev)
                    prev = (P, atb)
            mm2(*prev)

    if finalize:
        nc.finalize()
    return nc


_CACHE = {}


def _get_nc():
    if "nc" not in _CACHE:
        _CACHE["nc"] = build_nc()
    return _CACHE["nc"]


def make_in_maps(x, x_hat, h_dec_prev, W_read, b_read):
    import ml_dtypes
    bf16 = ml_dtypes.bfloat16
    x = np.asarray(x, np.float32)
    x_hat = np.asarray(x_hat, np.float32)
    h = np.asarray(h_dec_prev, np.float32)
    W = np.asarray(W_read, np.float32)
    bb = np.asarray(b_read, np.float32)

    # tiny params chain on host (2.6 MFLOP; the image relayout below is far
    # bigger): params -> per-(b,n) filterbank coefficients
    params = h @ W.T + bb                       # [256, 5]
    half = (IMG + 1) / 2.0
    g_x = half * (params[:, 0] + 1.0) - 0.5
    g_y = half * (params[:, 1] + 1.0) - 0.5
    s = np.exp(-0.5 * params[:, 2]) * np.float32(math.sqrt(0.5))  # 1/(sqrt2*sigma)
    delta = np.float32(DELTA_NORM) * np.exp(params[:, 3])
    gamma = np.exp(params[:, 4])
    offs = (np.arange(N, dtype=np.float32) - N / 2.0 + 0.5)
    mu_x = g_x[:, None] + offs[None, :] * delta[:, None]   # [256, N]
    mu_y = g_y[:, None] + offs[None, :] * delta[:, None]

    grid = np.broadcast_to(np.arange(IMG, dtype=np.float32), (128, IMG))

    def quadlay(a, ah):
        # 2x [32, C, H, W] f32 -> [quad, half, h, i, b2, c, w] bf16, contiguous
        q = np.stack([a, ah]).reshape(2, NT, 2, 2, C, IMG, IMG)
        q = q.transpose(1, 2, 5, 0, 3, 4, 6)
        return np.ascontiguousarray(q.astype(bf16)).reshape(
            NT * 2, 128, 2 * 2 * C * IMG)

    in_maps = []
    for i in range(NCORES):
        sl = slice(i * B, (i + 1) * B)
        s_l, g_l = s[sl], gamma[sl]
        bidx = np.arange(B * N) // N
        sbn = s_l[bidx]
        fbv = np.stack([
            sbn,
            -mu_x[sl].reshape(B * N) * sbn,
            -mu_y[sl].reshape(B * N) * sbn,
            g_l[bidx],
        ], axis=-1).astype(np.float32)                  # [B*N, 4]
        fbd = fbv.reshape(NT, 128, 4).transpose(1, 0, 2).reshape(128, NT * 4)
        cg = np.ascontiguousarray(np.concatenate([fbd, grid], axis=1))
        in_maps.append({
            "x4": quadlay(x[sl], x_hat[sl]),
            "cg": cg,
        })
    return in_maps


def unpack_out(o):
    """Device out [NT, 128, 2, 3, N] f32 -> [B, U*N*N] flattened reference layout."""
    o = np.asarray(o, np.float32)
    o = o.reshape(NT, 128, 2, 3, N).transpose(0, 2, 1, 3, 4).reshape(NP, 128, 3, N)
    full = np.empty((B, U * N * N), np.float32)
    a = o.reshape(NP, 4, N, 3, N)[:, :, :, 0:2, :]      # [P, ic, n, b2, m]
    full[:, :4 * N * N] = a.transpose(0, 3, 1, 2, 4).reshape(B, 4 * N * N)
    bpart = o.reshape(NP, 2, 2, N, 3, N)[:, :, :, :, 2, :]   # [P, b2, ic2, n, m]
    full[:, 4 * N * N:] = bpart.reshape(B, 2 * N * N)
    return full


def _install_ntff_hook():
    """The container's antenv package lacks axon_hooks; provide it so
    run_bass_kernel_spmd(trace=True) can capture an NTFF profile."""
    import sys
    import types
    if "antenv.axon_hooks" in sys.modules:
        return
    try:
        from trn_agent_boot.trn_boot import _ntff_profile_via_ctypes
    except ImportError:
        return
    mod = types.ModuleType("antenv.axon_hooks")
    hook = [_ntff_profile_via_ctypes("/opt/axon/libaxon_pjrt.so")]
    mod.set_axon_ntff_profile_hook = lambda h: hook.__setitem__(0, h)
    mod.get_axon_ntff_profile_hook = lambda: hook[0]
    sys.modules["antenv.axon_hooks"] = mod
    try:
        import antenv
        antenv.axon_hooks = mod
    except ImportError:
        pass


def run(inputs, trace=False, **spmd_kwargs):
    """Run on the 8 NeuronCores; returns (out [256, 6144] f32, BassKernelResults)."""
    if trace:
        _install_ntff_hook()
    nc = _get_nc()
    in_maps = make_in_maps(**inputs)
    res = run_bass_kernel_spmd(nc, in_maps, core_ids=list(range(NCORES)),
                               trace=trace, **spmd_kwargs)
    out = np.concatenate([unpack_out(res.results[i]["out"])
                          for i in range(NCORES)], axis=0)
    return out, res


def kernel(x, x_hat, h_dec_prev, W_read, b_read):
    out, _ = run(dict(x=x, x_hat=x_hat, h_dec_prev=h_dec_prev,
                      W_read=W_read, b_read=b_read))
    return out


# revision 12
# speedup vs baseline: 1.4949x; 1.4949x over previous
"""DRAW-style read attention on Trainium2 — data-parallel over batch on 8 NeuronCores.

reference math (per batch element):
    params = h @ W.T + b                         [5]
    g_x = 64.5*(p0+1)-0.5 ; g_y likewise
    sigma2 = exp(p2) ; delta = (127/31)*exp(p3) ; gamma = exp(p4)
    mu_x[n] = g_x + (n-15.5)*delta ; mu_y likewise
    FX[n,h] = exp(-(h-mu_x[n])^2/(2 sigma2)) / (Z_n + 1e-8)    (Z_n = row sum)
    FY[m,w] likewise
    patch_i = FX @ img_i @ FY.T   for img in (x_c0..2, xhat_c0..2)
    out = gamma * flatten(patches)               [6144]

device layout per core (local batch B=32):
    images arrive pre-cast to bf16 in [quad, h, b4, c, w] layout so every DMA
    partition line is a 3KB contiguous DRAM run; the 8 quad DMAs are the FIRST
    instructions on the sync queue so the HBM stream starts as early as
    possible (the kernel is HBM-stream-bound at ~358 GB/s/core)
    the tiny params chain (h @ W'' -> exp -> mu/s/gamma) runs on HOST; the
    device receives the per-(b,n) filterbank coefficients [s, -mu_x*s,
    -mu_y*s, gamma] directly (16KB), so the only ACT table ever loaded is
    erf_derivative and the whole on-device params/expansion pipeline is gone
    filterbanks in [bn, hw] layout: ACT Derivative_Erf(s*grid - mu*s) =
    2/sqrt(pi)*exp(-u^2) gives the gaussian + accum_out Z in one pass;
    normalize on DVE (gamma folded into FY's scale), then transpose to
    FXT/FYT [hw, bn] via REGULAR matmul against a bf16 identity (runs at
    2.4GHz vs 1.2GHz transpose-mode)
    main loop over pairs of b: At[w,n] = img[h,w].T @ FXT_b (12 matmuls/pair)
    then batched mm2: 4 matmuls/pair with At quads as 128/64-col stationary
    weights -> patch psum [128(ic,n), 32(m)]; outputs DMA'd in psum-native
    layout (384B lines, sync queue) and re-flattened on host
"""

import math

import numpy as np

import concourse.bass as bass  # noqa: F401  (import keeps bass registered)
import concourse.mybir as mybir
import concourse.tile as tile
from concourse import bacc
from concourse.bass_utils import run_bass_kernel_spmd
from concourse.masks import make_identity

F32 = mybir.dt.float32
BF16 = mybir.dt.bfloat16

NCORES = 8
B = 32          # per-core batch shard
C = 3
IMG = 128
N = 32
DH = 1024
U = 2 * C       # images per batch element: x channels 0..2 then x_hat channels 0..2
NT = (B * N) // 128   # quads: tiles over the flattened (b, n) axis
NP = B // 2           # batch pairs
DELTA_NORM = (IMG - 1.0) / (N - 1.0)
SQRT_PI_2 = math.sqrt(math.pi) / 2.0


def build_nc(finalize=True):
    nc = bacc.Bacc("TRN2", target_bir_lowering=False, debug=False, num_devices=NCORES)
    AFT = mybir.ActivationFunctionType

    # images pre-cast + pre-laid-out on host: [quad, h, i, b4, c, w] bf16
    # (x and x_hat interleaved so every quad is ONE 6KB-per-line DMA)
    x4_d = nc.declare_dram_parameter("x4", [NT, 128, 2 * 4 * C * IMG], BF16,
                                     isOutput=False)
    # per-(b,n) filterbank coefficients [s, -mu_x*s, -mu_y*s, gamma] + grid
    cg_d = nc.declare_dram_parameter("cg", [128, NT * 4 + IMG], F32,
                                     isOutput=False)
    # psum-native output layout, bf16 (host re-flattens + upcasts, see
    # unpack_out; the rel-err budget easily covers one more bf16 rounding)
    out_d = nc.declare_dram_parameter("out", [NT, 128, 2, 3, N], BF16,
                                      isOutput=True)

    with tile.TileContext(nc) as tc:
        with (
            tc.tile_pool(name="consts", bufs=1) as consts,
            tc.tile_pool(name="fb", bufs=4) as fb,
            tc.tile_pool(name="imgb_p", bufs=8) as imgb_p,
            tc.tile_pool(name="atb_p", bufs=3) as atb_p,
            tc.tile_pool(name="outs_p", bufs=3) as outs_p,
            tc.tile_pool(name="ps_tr", bufs=2, space="PSUM") as ps_tr,
            tc.tile_pool(name="ps_at", bufs=3, space="PSUM") as ps_at,
            tc.tile_pool(name="ps_pt", bufs=2, space="PSUM") as ps_pt,
        ):
            # ---- filterbank coefficients FIRST on the sync queue (NOT the
            # scalar queue: the erf_derivative ACT table load is itself a DMA
            # on the scalar HWDGE ring and would push this transfer out by
            # ~3us), then the image stream
            cg_sb = consts.tile([128, NT * 4 + IMG], F32)
            nc.sync.dma_start(out=cg_sb, in_=cg_d[:])
            fbv = cg_sb[:, 0:NT * 4].rearrange("p (t j) -> p t j", t=NT)
            grid_sb = cg_sb[:, NT * 4:]

            def emit_quad_dma(t):
                imgb = imgb_p.tile([128, 2, 4, C, IMG], BF16, tag="imgb",
                                   name=f"imgb{t}")
                nc.sync.dma_start(out=imgb, in_=x4_d[t])
                return imgb

            quads = {t: emit_quad_dma(t) for t in range(NT)}

            # tiny on-chip consts + ACT table prime while DMAs fly (memset
            # before make_identity so the prime's bias operand is ready
            # immediately and the ~2.9us table load starts ASAP)
            zeros = consts.tile([128, 1], F32)
            nc.vector.memset(zeros, 0.0)
            prime_t = consts.tile([1, 1], F32)
            nc.scalar.activation(prime_t, zeros[:1], AFT.Derivative_Erf,
                                 scale=-1.0, bias=zeros[:1])
            ident = consts.tile([128, 128], BF16)
            make_identity(nc, ident)

            # both filterbanks bf16 (matmuls run bf16); gamma folded into FY;
            # one combined tile so each tile's epilogue is a single DVE cast
            FXY = consts.tile([128, 2, B * N], BF16)
            FXT = FXY[:, 0, :]
            FYT = FXY[:, 1, :]

            def fbank2(t):
                # Derivative_Erf(u) = 2/sqrt(pi) * exp(-u^2) with u =
                # s*grid - s*mu: one ACT per filterbank half, accum_out
                # yields Z for free. The 2/sqrt(pi) factor cancels in
                # F/(Z+eps) once eps is scaled by the same factor.
                e_un = fb.tile([128, 2, IMG], BF16, tag="e_un")
                Z2 = fb.tile([128, 2], F32, tag="Z2")
                nc.scalar.activation(e_un[:, 0, :], grid_sb, AFT.Derivative_Erf,
                                     scale=fbv[:, t, 0:1], bias=fbv[:, t, 1:2],
                                     accum_out=Z2[:, 0:1])
                nc.scalar.activation(e_un[:, 1, :], grid_sb, AFT.Derivative_Erf,
                                     scale=fbv[:, t, 0:1], bias=fbv[:, t, 2:3],
                                     accum_out=Z2[:, 1:2])
                # the reference ADDS eps — for borderline off-grid rows Z is
                # itself ~1e-8, so a max-clamp is NOT equivalent. Startup
                # tiles run the chain on DVE (fewer cross-engine hops);
                # steady-state puts the small ops on the idle gpsimd
                small = nc.vector if t < 2 else nc.gpsimd
                small.tensor_scalar_add(Z2, Z2, 1e-8 / SQRT_PI_2)
                invZ2 = fb.tile([128, 2], F32, tag="invZ2")
                nc.vector.reciprocal_approx_fast(invZ2, Z2)
                small.tensor_mul(invZ2[:, 1:2], invZ2[:, 1:2], fbv[:, t, 3:4])
                ps_t = ps_tr.tile([128, 2, 128], F32, tag="ps_t")
                for j in range(2):
                    Fn = fb.tile([128, IMG], BF16, tag="Fn")
                    nc.vector.tensor_scalar_mul(Fn, e_un[:, j, :],
                                                invZ2[:, j:j + 1])
                    # Fn.T @ I — a regular matmul IS the transpose, and runs
                    # at the warm 2.4GHz clock (transpose-mode stays at 1.2)
                    nc.tensor.matmul(ps_t[:, j, :], Fn, ident,
                                     start=True, stop=True)
                nc.vector.tensor_copy(FXY[:, :, t * 128:(t + 1) * 128], ps_t)

            # ---- main loop: pairs of batch elements; mm2 pipelined one pair
            # behind so the atb copy latency hides under the next mm1 ----
            def mm1(P, imgb, pp):
                ps_a = ps_at.tile([128, 2, U, N], F32, tag="ps_a")
                for b2 in range(2):
                    b = 2 * P + b2
                    for i in range(2):
                        for c in range(C):
                            nc.tensor.matmul(ps_a[:, b2, i * C + c, :],
                                             imgb[:, i, 2 * pp + b2, c, :],
                                             FXT[:, b * N:(b + 1) * N],
                                             start=True, stop=True)
                atb = atb_p.tile([128, 2, U, N], BF16, tag="atb")
                nc.vector.tensor_copy(atb, ps_a)
                return atb

            tile_ps = {}

            def mm2(P, atb):
                # At quads as 128/64-col stationary weights: 4 matmuls/pair.
                # psum partitions: j0/j1 -> (ic0..3, n) of b0/b1, j2 -> (b2,
                # ic4..5, n) via the 64-col tile at column offset 64.
                # both pairs of a quad share one psum tile so the epilogue is
                # a single copy + DMA per quad
                tq, pp = P // 2, P % 2
                if pp == 0:
                    tile_ps[tq] = ps_pt.tile([128, 2, 3, N], F32, tag="ps_o",
                                             name=f"pso{tq}")
                ps_p = tile_ps[tq]
                b0, b1 = 2 * P, 2 * P + 1
                fy0 = FYT[:, b0 * N:(b0 + 1) * N]
                fy1 = FYT[:, b1 * N:(b1 + 1) * N]
                nc.tensor.matmul(ps_p[:, pp, 0, :], atb[:, 0, 0:4, :], fy0,
                                 start=True, stop=True)
                nc.tensor.matmul(ps_p[:, pp, 1, :], atb[:, 1, 0:4, :], fy1,
                                 start=True, stop=True)
                nc.tensor.matmul(ps_p[0:64, pp, 2, :], atb[:, 0, 4:6, :], fy0,
                                 start=True, stop=True)
                nc.tensor.matmul(ps_p[64:128, pp, 2, :], atb[:, 1, 4:6, :], fy1,
                                 start=True, stop=True, tile_position=(0, 64))
                if pp == 1:
                    outs = outs_p.tile([128, 2, 3, N], BF16, tag="outs")
                    nc.scalar.copy(outs, tile_ps.pop(tq))
                    nc.sync.dma_start(out=out_d[tq], in_=outs)

            fbank2(0)
            fbank2(1)
            prev = None
            fbank2(2)
            for t in range(NT):
                # filterbanks three tiles ahead
                if t + 3 < NT:
                    fbank2(t + 3)
                imgb = quads.pop(t)
                for pp in range(2):
                    P = 2 * t + pp
                    atb = mm1(P, imgb, pp)
                    if prev is not None:
                        mm2(*prev)
                    prev = (P, atb)
            mm2(*prev)

    if finalize:
        nc.finalize()
    return nc


_CACHE = {}


def _get_nc():
    if "nc" not in _CACHE:
        _CACHE["nc"] = build_nc()
    return _CACHE["nc"]


def make_in_maps(x, x_hat, h_dec_prev, W_read, b_read):
    import ml_dtypes
    bf16 = ml_dtypes.bfloat16
    x = np.asarray(x, np.float32)
    x_hat = np.asarray(x_hat, np.float32)
    h = np.asarray(h_dec_prev, np.float32)
    W = np.asarray(W_read, np.float32)
    bb = np.asarray(b_read, np.float32)

    # tiny params chain on host (2.6 MFLOP; the image relayout below is far
    # bigger): params -> per-(b,n) filterbank coefficients
    params = h @ W.T + bb                       # [256, 5]
    half = (IMG + 1) / 2.0
    g_x = half * (params[:, 0] + 1.0) - 0.5
    g_y = half * (params[:, 1] + 1.0) - 0.5
    s = np.exp(-0.5 * params[:, 2]) * np.float32(math.sqrt(0.5))  # 1/(sqrt2*sigma)
    delta = np.float32(DELTA_NORM) * np.exp(params[:, 3])
    gamma = np.exp(params[:, 4])
    offs = (np.arange(N, dtype=np.float32) - N / 2.0 + 0.5)
    mu_x = g_x[:, None] + offs[None, :] * delta[:, None]   # [256, N]
    mu_y = g_y[:, None] + offs[None, :] * delta[:, None]

    grid = np.broadcast_to(np.arange(IMG, dtype=np.float32), (128, IMG))

    def quadlay(a, ah):
        # 2x [32, C, H, W] f32 -> [quad, h, i, b4, c, w] bf16, contiguous
        q = np.stack([a, ah]).reshape(2, NT, 4, C, IMG, IMG)
        q = q.transpose(1, 4, 0, 2, 3, 5)
        return np.ascontiguousarray(q.astype(bf16)).reshape(
            NT, 128, 2 * 4 * C * IMG)

    in_maps = []
    for i in range(NCORES):
        sl = slice(i * B, (i + 1) * B)
        s_l, g_l = s[sl], gamma[sl]
        bidx = np.arange(B * N) // N
        sbn = s_l[bidx]
        fbv = np.stack([
            sbn,
            -mu_x[sl].reshape(B * N) * sbn,
            -mu_y[sl].reshape(B * N) * sbn,
            g_l[bidx],
        ], axis=-1).astype(np.float32)                  # [B*N, 4]
        fbd = fbv.reshape(NT, 128, 4).transpose(1, 0, 2).reshape(128, NT * 4)
        cg = np.ascontiguousarray(np.concatenate([fbd, grid], axis=1))
        in_maps.append({
            "x4": quadlay(x[sl], x_hat[sl]),
            "cg": cg,
        })
    return in_maps


def unpack_out(o):
    """Device out [NT, 128, 2, 3, N] f32 -> [B, U*N*N] flattened reference layout."""
    o = np.asarray(o, np.float32)
    o = o.reshape(NT, 128, 2, 3, N).transpose(0, 2, 1, 3, 4).reshape(NP, 128, 3, N)
    full = np.empty((B, U * N * N), np.float32)
    a = o.reshape(NP, 4, N, 3, N)[:, :, :, 0:2, :]      # [P, ic, n, b2, m]
    full[:, :4 * N * N] = a.transpose(0, 3, 1, 2, 4).reshape(B, 4 * N * N)
    bpart = o.reshape(NP, 2, 2, N, 3, N)[:, :, :, :, 2, :]   # [P, b2, ic2, n, m]
    full[:, 4 * N * N:] = bpart.reshape(B, 2 * N * N)
    return full


def _install_ntff_hook():
    """The container's antenv package lacks axon_hooks; provide it so
    run_bass_kernel_spmd(trace=True) can capture an NTFF profile."""
    import sys
    import types
    if "antenv.axon_hooks" in sys.modules:
        return
    try:
        from trn_agent_boot.trn_boot import _ntff_profile_via_ctypes
    except ImportError:
        return
    mod = types.ModuleType("antenv.axon_hooks")
    hook = [_ntff_profile_via_ctypes("/opt/axon/libaxon_pjrt.so")]
    mod.set_axon_ntff_profile_hook = lambda h: hook.__setitem__(0, h)
    mod.get_axon_ntff_profile_hook = lambda: hook[0]
    sys.modules["antenv.axon_hooks"] = mod
    try:
        import antenv
        antenv.axon_hooks = mod
    except ImportError:
        pass


def run(inputs, trace=False, **spmd_kwargs):
    """Run on the 8 NeuronCores; returns (out [256, 6144] f32, BassKernelResults)."""
    if trace:
        _install_ntff_hook()
    nc = _get_nc()
    in_maps = make_in_maps(**inputs)
    res = run_bass_kernel_spmd(nc, in_maps, core_ids=list(range(NCORES)),
                               trace=trace, **spmd_kwargs)
    out = np.concatenate([unpack_out(res.results[i]["out"])
                          for i in range(NCORES)], axis=0)
    return out, res


def kernel(x, x_hat, h_dec_prev, W_read, b_read):
    out, _ = run(dict(x=x, x_hat=x_hat, h_dec_prev=h_dec_prev,
                      W_read=W_read, b_read=b_read))
    return out
